# revision 1
# baseline (speedup 1.0000x reference)
"""2-layer GCN (nn_Discriminator2) on 8 Trainium2 NeuronCores via Bass/Tile.

Decomposition (dest-sharded graph parallel):
  conv1: h = x @ W1 computed locally per node shard (feature-transposed lhsT),
         pre-scaled by dis (h~ = dis * h), written padded-bf16, AllGathered.
         Aggregation: edges sorted by dest, diced into 128-edge tiles inside
         fixed 32-dest windows; each tile is a PE matmul
         psum[w*32:(w+1)*32] += S_tile.T @ msg_tile with binary S (host-built)
         and msg rows fetched by dma_gather from the AllGathered h~ table.
  conv2: (A @ h1) @ W2 instead of A @ (h1 @ W2): reuses the exact same
         edge structure/S/indices on h1~ = dis * h1, then a fused
         tensor_tensor_reduce dot with W2, BN2, relu, sigmoid.

SPMD constraint: one instruction stream for all 8 cores -> the tile structure
(T[b][w] counts) is maxed across cores; cores pad with all-zero S rows.
"""

import math
import numpy as np
import ml_dtypes

BF16 = ml_dtypes.bfloat16
EPS = 1e-3
P = 128          # partitions / dest-block size
WIN = 32         # dest window (matmul M)
NWIN = P // WIN


# ----------------------------------------------------------------------------
# Host-side graph preprocessing (structure only: indices, binary selectors)
# ----------------------------------------------------------------------------

def preprocess(edge_index: np.ndarray, n: int, ncores: int):
    """Balanced packing: permute dest nodes across (core, block, window) bins so
    per-bin edge counts are near-equal; the SPMD cross-core max then adds ~no
    padding. pos[v] = global slot of node v; all device arrays follow pos."""
    src = edge_index[0].astype(np.int64)
    dst = edge_index[1].astype(np.int64)
    deg = np.bincount(dst, minlength=n).astype(np.int64) + 1   # incl self-loop
    dis = (1.0 / np.sqrt(deg.astype(np.float64))).astype(np.float32)

    nblk = max(1, math.ceil(n / ncores / P))
    nlocp = nblk * P
    nbin_core = nblk * NWIN
    nbins = ncores * nbin_core

    # deal nodes (ranked by degree desc) snake-wise across bins, cap WIN each
    order = np.argsort(-deg, kind="stable")
    binload = np.zeros(nbins, np.int64)
    bincnt = np.zeros(nbins, np.int64)
    slot_of = np.zeros(n, np.int64)     # global position of node v
    bin_members = [[] for _ in range(nbins)]
    bi = 0
    direction = 1
    for v in order:
        # skip full bins
        tries = 0
        while bincnt[bi] >= WIN:
            bi += direction
            if bi == nbins:
                bi = nbins - 1; direction = -1
            elif bi < 0:
                bi = 0; direction = 1
            tries += 1
            assert tries <= 2 * nbins
        bin_members[bi].append(v)
        bincnt[bi] += 1
        binload[bi] += deg[v]
        bi += direction
        if bi == nbins:
            bi = nbins - 1; direction = -1
        elif bi < 0:
            bi = 0; direction = 1
    for b in range(nbins):
        core, rem = divmod(b, nbin_core)
        blk, win = divmod(rem, NWIN)
        base = core * nlocp + blk * P + win * WIN
        for s_, v in enumerate(bin_members[b]):
            slot_of[v] = base + s_

    loops = np.arange(n, dtype=np.int64)
    s_all = np.concatenate([src, loops])
    d_all = np.concatenate([dst, loops])
    dpos = slot_of[d_all]               # dest slot position
    spos = slot_of[s_all]               # source row in the h~ table
    core = dpos // nlocp
    rem = dpos - core * nlocp
    blk = rem // P
    win = (rem % P) // WIN
    wloc = rem % WIN

    cnt = np.zeros((ncores, nblk, NWIN), np.int64)
    np.add.at(cnt, (core, blk, win), 1)
    T = np.maximum(1, -(-cnt // P)).max(axis=0)          # [nblk, NWIN]
    tb = T.sum(axis=1)
    slot_base_bw = np.zeros((nblk, NWIN), np.int64)
    flat = T.reshape(-1)
    slot_base_bw.reshape(-1)[1:] = np.cumsum(flat)[:-1]
    tot = int(flat.sum())

    # order edges by (core, blk, win); sequence within group -> tile/lane
    key = (core * nblk + blk) * NWIN + win
    order_e = np.argsort(key, kind="stable")
    c_o, b_o, w_o, wl_o, sp_o = (core[order_e], blk[order_e], win[order_e],
                                 wloc[order_e], spos[order_e])
    key_o = key[order_e]
    first = np.r_[True, key_o[1:] != key_o[:-1]]
    idx_arr = np.arange(len(key_o))
    grp_start = np.maximum.accumulate(np.where(first, idx_arr, 0))
    seq = idx_arr - grp_start
    tile_k = seq // P
    jj = seq % P
    slot = slot_base_bw[b_o, w_o] + tile_k
    assert (tile_k < T[b_o, w_o]).all()

    idx16 = np.zeros((ncores, tot * P), np.int16)
    idx16[c_o, slot * P + jj] = sp_o.astype(np.int16)
    S = np.zeros((ncores, P, tot * WIN), BF16)
    S[c_o, jj, slot * WIN + wl_o] = BF16(1.0)

    # wrapped int16 index layout: slot-major i -> [i % 16, i // 16],
    # replicated into all 8 GPSIMD core partition groups (HW reads per-core)
    idxw = np.tile(idx16.reshape(ncores, tot * 8, 16).transpose(0, 2, 1),
                   (1, 8, 1)).copy()

    return dict(
        dis=dis, nloc=nlocp, nblk=nblk, nlocp=nlocp, tot=tot,
        T=T, tb=tb, idxw=idxw, S=S, slot_of=slot_of,
    )


# ----------------------------------------------------------------------------
# Bass program
# ----------------------------------------------------------------------------

def build_program(n, f, ncores, nblk, nlocp, tot, T, tb, nloc):
    import concourse.bacc as bacc
    import concourse.mybir as mybir
    import concourse.tile as tile

    fpad = -(-f // P) * P          # bf16 row padded so 2*fpad % 256 == 0
    kch = fpad // P                # contraction chunks for x @ W1
    ag_rows = ncores * nlocp
    dt = mybir.dt
    Alu = mybir.AluOpType
    Act = mybir.ActivationFunctionType

    nc = bacc.Bacc("TRN2", target_bir_lowering=False, debug=False,
                   num_devices=ncores)

    xT_in = nc.dram_tensor("xT", [kch * P, nlocp], dt.bfloat16, kind="ExternalInput")
    W1_in = nc.dram_tensor("W1p", [kch * P, f], dt.bfloat16, kind="ExternalInput")
    vecs_in = nc.dram_tensor("vecs", [6, f], dt.float32, kind="ExternalInput")
    scal_in = nc.dram_tensor("scal", [1, 8], dt.float32, kind="ExternalInput")
    dis_in = nc.dram_tensor("disb", [P, nblk], dt.float32, kind="ExternalInput")
    disw_in = nc.dram_tensor("disw", [WIN, nblk * NWIN], dt.float32, kind="ExternalInput")
    idx_in = nc.dram_tensor("idxw", [P, tot * 8], dt.int16, kind="ExternalInput")
    S_in = nc.dram_tensor("S", [P, tot * WIN], dt.bfloat16, kind="ExternalInput")
    out_ext = nc.dram_tensor("out", [nloc, 1], dt.float32, kind="ExternalOutput")

    shared = "Shared" if ncores > 4 else "Local"
    h_loc = nc.dram_tensor("h_loc", [nlocp, fpad], dt.bfloat16)
    h_ag = nc.dram_tensor("h_ag", [ag_rows, fpad], dt.bfloat16, addr_space=shared)
    h1_loc = nc.dram_tensor("h1_loc", [nlocp, fpad], dt.bfloat16)
    h1_ag = nc.dram_tensor("h1_ag", [ag_rows, fpad], dt.bfloat16, addr_space=shared)

    rg = [list(range(ncores))]

    with tile.TileContext(nc) as tc:
        with (
            tc.tile_pool(name="const", bufs=1) as cp,
            tc.tile_pool(name="work", bufs=3) as wp,
            tc.tile_pool(name="msgp", bufs=2) as mp,
            tc.tile_pool(name="psum", bufs=8, space="PSUM") as pp,
        ):
            # ---------------- constants ----------------
            xT_sb = cp.tile([P, kch, nlocp], dt.bfloat16)
            nc.sync.dma_start(out=xT_sb[:], in_=xT_in.ap().rearrange("(k p) n -> p k n", p=P))
            W1_sb = cp.tile([P, kch, f], dt.bfloat16)
            nc.sync.dma_start(out=W1_sb[:], in_=W1_in.ap().rearrange("(k p) n -> p k n", p=P))
            scal = cp.tile([1, 8], dt.float32)
            nc.sync.dma_start(out=scal[:], in_=scal_in[:])
            dis_sb = cp.tile([P, nblk], dt.float32)
            nc.sync.dma_start(out=dis_sb[:], in_=dis_in[:])
            disw_sb = cp.tile([WIN, nblk * NWIN], dt.float32)
            nc.sync.dma_start(out=disw_sb[:], in_=disw_in[:])
            idx_sb = cp.tile([P, tot * 8], dt.int16)
            nc.sync.dma_start(out=idx_sb[:], in_=idx_in[:])
            S_sb = cp.tile([P, tot * WIN], dt.bfloat16)
            nc.sync.dma_start(out=S_sb[:], in_=S_in[:])

            ones = cp.tile([1, P], dt.float32)
            nc.vector.memset(ones[:], 1.0)

            # zero the padded tail columns of the gather tables once
            zpad = cp.tile([P, fpad - f], dt.bfloat16)
            nc.vector.memset(zpad[:], 0.0)
            for nb in range(nblk):
                nc.sync.dma_start(out=h_loc[nb * P:(nb + 1) * P, f:], in_=zpad[:])
                nc.sync.dma_start(out=h1_loc[nb * P:(nb + 1) * P, f:], in_=zpad[:])

            # copy each param row to its own partition-0 tile (matmul rhs and
            # DVE operands need base partition 0)
            vrow = []
            for i in range(6):
                r = cp.tile([1, f], dt.float32, tag=f"vrow{i}")
                nc.sync.dma_start(out=r[:], in_=vecs_in[i:i + 1, :])
                vrow.append(r)

            # k1 = g1 / sqrt(rv1 + eps); t1 = beta1 - rm1 * k1      [1, f]
            k1 = cp.tile([1, f], dt.float32)
            t1 = cp.tile([1, f], dt.float32)
            tmp = cp.tile([1, f], dt.float32)
            nc.vector.tensor_scalar_add(tmp[:], vrow[4][:], EPS)
            nc.scalar.sqrt(tmp[:], tmp[:])
            nc.vector.reciprocal(tmp[:], tmp[:])
            nc.vector.tensor_tensor(out=k1[:], in0=tmp[:], in1=vrow[1][:], op=Alu.mult)
            nc.vector.tensor_tensor(out=tmp[:], in0=vrow[3][:], in1=k1[:], op=Alu.mult)
            nc.vector.tensor_tensor(out=t1[:], in0=vrow[2][:], in1=tmp[:], op=Alu.subtract)

            # k2 = g2 / sqrt(rv2 + eps); t2 = beta2 - rm2 * k2; pack [1,4]: b2,k2,t2
            sc_row = cp.tile([1, 4], dt.float32)
            nc.vector.memset(sc_row[:], 0.0)
            stmp = cp.tile([1, 1], dt.float32)
            nc.vector.tensor_copy(out=sc_row[:, 0:1], in_=scal[:, 0:1])           # b2
            nc.vector.tensor_scalar_add(stmp[:], scal[:, 4:5], EPS)
            nc.scalar.sqrt(stmp[:], stmp[:])
            nc.vector.reciprocal(stmp[:], stmp[:])
            nc.vector.tensor_tensor(out=sc_row[:, 1:2], in0=stmp[:], in1=scal[:, 1:2], op=Alu.mult)  # k2
            nc.vector.tensor_tensor(out=stmp[:], in0=scal[:, 3:4], in1=sc_row[:, 1:2], op=Alu.mult)
            nc.vector.tensor_tensor(out=sc_row[:, 2:3], in0=scal[:, 2:3], in1=stmp[:], op=Alu.subtract)  # t2

            # replicate rows across 128 partitions via ones-matmul
            def replicate(row_ap, width):
                ps = pp.tile([P, width], dt.float32, tag="ps")
                nc.tensor.matmul(out=ps[:], lhsT=ones[:], rhs=row_ap, start=True, stop=True)
                sb = cp.tile([P, width], dt.float32, tag=f"rep{replicate.i}")
                replicate.i += 1
                nc.vector.tensor_copy(out=sb[:], in_=ps[:])
                return sb
            replicate.i = 0

            B1rep = replicate(vrow[0][:], f)
            K1rep = replicate(k1[:], f)
            T1rep = replicate(t1[:], f)
            W2rep = replicate(vrow[5][:], f)
            SCrep = replicate(sc_row[:], 4)      # cols: b2, k2, t2

            # ---------------- phase 1: h~ = dis * (x @ W1) ----------------
            for nb in range(nblk):
                ps = pp.tile([P, f], dt.float32, tag="ps")
                for kc in range(kch):
                    nc.tensor.matmul(
                        out=ps[:],
                        lhsT=xT_sb[:, kc, nb * P:(nb + 1) * P],
                        rhs=W1_sb[:, kc, :],
                        start=(kc == 0), stop=(kc == kch - 1),
                    )
                hb = wp.tile([P, f], dt.bfloat16, tag="hb")
                nc.vector.tensor_scalar_mul(hb[:], ps[:], dis_sb[:, nb:nb + 1])
                nc.sync.dma_start(out=h_loc[nb * P:(nb + 1) * P, :f], in_=hb[:])

            nc.gpsimd.collective_compute(
                "AllGather", Alu.bypass, replica_groups=rg,
                ins=[h_loc[:]], outs=[h_ag[:]],
            )

            # ---------------- aggregation pass (shared by both layers) -----
            def aggregate(b, src_dram):
                base = int(T[:b].sum()) if b else 0
                ntile = int(tb[b])
                msg = mp.tile([P, ntile, fpad], dt.bfloat16, tag="msg")
                nc.gpsimd.dma_gather(
                    out_ap=msg[:],
                    in_ap=src_dram[:],
                    idxs_ap=idx_sb[:, base * 8:(base + ntile) * 8],
                    num_idxs=ntile * P,
                    num_idxs_reg=ntile * P,
                    elem_size=fpad,
                    single_packet=False,
                )
                pws = []
                slot = base
                for w in range(NWIN):
                    tw = int(T[b, w])
                    pw = pp.tile([WIN, f], dt.float32, tag="ps")
                    for k in range(tw):
                        nc.tensor.matmul(
                            out=pw[:],
                            lhsT=S_sb[:, slot * WIN:(slot + 1) * WIN],
                            rhs=msg[:, slot - base, :f],
                            start=(k == 0), stop=(k == tw - 1),
                        )
                        slot += 1
                    pws.append(pw)
                return pws

            # ---------------- conv1 epilogue -> h1~ ----------------
            for b in range(nblk):
                pws = aggregate(b, h_ag)
                for w in range(NWIN):
                    dw = disw_sb[:, b * NWIN + w:b * NWIN + w + 1]
                    u = wp.tile([WIN, f], dt.float32, tag="u")
                    nc.vector.tensor_scalar_mul(u[:], pws[w][:], dw)
                    nc.vector.tensor_tensor(out=u[:], in0=u[:], in1=B1rep[:WIN, :], op=Alu.add)
                    nc.scalar.activation(u[:], u[:], Act.Relu)
                    nc.vector.tensor_tensor(out=u[:], in0=u[:], in1=K1rep[:WIN, :], op=Alu.mult)
                    nc.vector.tensor_tensor(out=u[:], in0=u[:], in1=T1rep[:WIN, :], op=Alu.add)
                    nc.scalar.activation(u[:], u[:], Act.Relu)
                    h1b = wp.tile([WIN, f], dt.bfloat16, tag="hb")
                    nc.vector.tensor_scalar_mul(h1b[:], u[:], dw)
                    nc.sync.dma_start(out=h1_loc[b * P + w * WIN:b * P + (w + 1) * WIN, :f], in_=h1b[:])

            nc.gpsimd.collective_compute(
                "AllGather", Alu.bypass, replica_groups=rg,
                ins=[h1_loc[:]], outs=[h1_ag[:]],
            )

            # ---------------- conv2 ----------------
            for b in range(nblk):
                pws = aggregate(b, h1_ag)
                for w in range(NWIN):
                    r0 = b * P + w * WIN
                    rows = min(WIN, nloc - r0)
                    if rows <= 0:
                        continue
                    dw = disw_sb[:, b * NWIN + w:b * NWIN + w + 1]
                    sc = wp.tile([WIN, f], dt.float32, tag="u")
                    z = wp.tile([WIN, 1], dt.float32, tag="z")
                    nc.vector.tensor_tensor(out=sc[:], in0=pws[w][:], in1=W2rep[:WIN, :], op=Alu.mult)
                    nc.vector.tensor_reduce(out=z[:], in_=sc[:], axis=mybir.AxisListType.X, op=Alu.add)
                    nc.vector.tensor_scalar_mul(z[:], z[:], dw)
                    nc.vector.tensor_tensor(out=z[:], in0=z[:], in1=SCrep[:WIN, 0:1], op=Alu.add)
                    nc.vector.tensor_tensor(out=z[:], in0=z[:], in1=SCrep[:WIN, 1:2], op=Alu.mult)
                    nc.vector.tensor_tensor(out=z[:], in0=z[:], in1=SCrep[:WIN, 2:3], op=Alu.add)
                    o = wp.tile([WIN, 1], dt.float32, tag="o")
                    nc.scalar.activation(o[:], z[:], Act.Relu)
                    nc.scalar.activation(o[:], o[:], Act.Sigmoid)
                    nc.sync.dma_start(out=out_ext[r0:r0 + rows, :], in_=o[:rows, :])

    nc.compile()
    return nc


# ----------------------------------------------------------------------------
# Full pipeline
# ----------------------------------------------------------------------------

def make_inputs(x, W1, b1, g1, beta1, rm1, rv1, W2, b2, g2, beta2, rm2, rv2,
                pre, ncores):
    n, f = x.shape
    nlocp, nblk = pre["nlocp"], pre["nblk"]
    slot_of = pre["slot_of"]
    kch = -(-f // P)
    fpk = kch * P

    W1p = np.zeros((fpk, f), BF16)
    W1p[:f, :] = W1.astype(BF16)
    vecs = np.stack([b1, g1, beta1, rm1, rv1, W2[:, 0]]).astype(np.float32)
    scal = np.zeros((1, 8), np.float32)
    scal[0, :5] = [b2[0], g2[0], beta2[0], rm2[0], rv2[0]]

    dis = pre["dis"]
    core_of = slot_of // nlocp
    local = slot_of - core_of * nlocp
    in_maps = []
    for c in range(ncores):
        sel = core_of == c
        loc = local[sel]
        xT = np.zeros((fpk, nlocp), BF16)
        xT[:f, loc] = x[sel].T.astype(BF16)
        db = np.zeros(nlocp, np.float32)
        db[loc] = dis[sel]
        disb = db.reshape(nblk, P).T.copy()
        disw = db.reshape(nblk * (P // WIN), WIN).T.copy()
        in_maps.append({
            "xT": xT, "W1p": W1p, "vecs": vecs, "scal": scal,
            "disb": disb, "disw": disw, "idxw": pre["idxw"][c], "S": pre["S"][c],
        })
    return in_maps


def _install_ntff_hook():
    """bass_utils wants antenv.axon_hooks for trace=True under axon; this
    container's antenv lacks it. Inject a shim backed by the boot helper."""
    import sys, types
    if "antenv.axon_hooks" in sys.modules:
        return
    try:
        from trn_agent_boot.trn_boot import _ntff_profile_via_ctypes
        hook = _ntff_profile_via_ctypes("/opt/axon/libaxon_pjrt.so")
    except Exception:
        hook = None
    mod = types.ModuleType("antenv.axon_hooks")
    mod.get_axon_ntff_profile_hook = lambda: hook
    mod.set_axon_ntff_profile_hook = lambda h: None
    sys.modules["antenv.axon_hooks"] = mod


def run(inputs, ncores=8, trace=False, tmpdir=None):
    from concourse.bass_utils import run_bass_kernel_spmd
    if trace:
        _install_ntff_hook()

    x = np.asarray(inputs["x"])
    n, f = x.shape
    pre = preprocess(np.asarray(inputs["edge_index"]), n, ncores)
    nc = build_program(n, f, ncores, pre["nblk"], pre["nlocp"], pre["tot"],
                       pre["T"], pre["tb"], pre["nloc"])
    in_maps = make_inputs(
        x, *(np.asarray(inputs[k]) for k in
             ["W1", "b1", "g1", "beta1", "rm1", "rv1",
              "W2", "b2", "g2", "beta2", "rm2", "rv2"]),
        pre, ncores)
    res = run_bass_kernel_spmd(nc, in_maps, list(range(ncores)), trace=trace,
                               tmpdir=tmpdir)
    allout = np.concatenate([res.results[c]["out"] for c in range(ncores)], axis=0)
    out = allout[pre["slot_of"]]
    return out, res, pre, nc


# ----------------------------------------------------------------------------
# Harness entry point: full inputs in, full output out.
# ----------------------------------------------------------------------------

_CACHE = {}


def kernel(**inputs) -> np.ndarray:
    out, _res, _pre, _nc = run(inputs, ncores=8, trace=False)
    return out.astype(np.float32)



# revision 3
# speedup vs baseline: 1.0783x; 1.0783x over previous
"""2-layer GCN (nn_Discriminator2) on 8 Trainium2 NeuronCores via Bass/Tile.

Design S (v1):
  conv1: h~ = dis * (x @ W1) per node shard (bf16, padded row), AllGather.
         Aggregation: edges sorted by dest, 128-edge tiles in 32-dest windows;
         PE matmul psum[128-dest block] += S_tile.T @ msg_tile with msg rows
         fetched by dma_gather. Epilogue per 128-dest block fused on DVE:
         u = relu(BN1(relu(dis_d*psum + b1))), then z = dis_d * (u @ W2)
         computed immediately (DVE mult+reduce) -- h1 rows are never stored.
  conv2: AllGather z (80KB instead of 15.7MB h1), build a replicated z table
         in SBUF (gpsimd partition_broadcast), gather z[src] per edge with
         ap_gather (8-way split across GPSIMD core groups, per-dest padded),
         DVE segmented reduce per window, scalar epilogue, sigmoid.
"""

import math
import numpy as np
import ml_dtypes

BF16 = ml_dtypes.bfloat16
EPS = 1e-3
P = 128          # partitions / dest-block size
WIN = 32         # dest window (matmul M)
NWIN = P // WIN
NGRP = 8         # GPSIMD core groups for conv2 scalar gather


# ----------------------------------------------------------------------------
# Host-side graph preprocessing (structure only: indices, binary selectors)
# ----------------------------------------------------------------------------

def preprocess(edge_index: np.ndarray, n: int, ncores: int):
    """Balanced packing: permute dest nodes across (core, block, window) bins so
    per-bin edge counts are near-equal; the SPMD cross-core max then adds ~no
    padding. pos[v] = global slot of node v; all device arrays follow pos."""
    src = edge_index[0].astype(np.int64)
    dst = edge_index[1].astype(np.int64)
    deg = np.bincount(dst, minlength=n).astype(np.int64) + 1   # incl self-loop
    dis = (1.0 / np.sqrt(deg.astype(np.float64))).astype(np.float32)

    nblk = max(1, math.ceil(n / ncores / P))
    nlocp = nblk * P
    nbin_core = nblk * NWIN
    nbins = ncores * nbin_core

    # bins = 32 CONSECUTIVE degree ranks each (uniform degree within a bin ->
    # tight per-dest padding for conv2); snake-deal whole bins across cores by
    # bin load (conv1 cross-core tile padding stays small); each core's bins
    # are placed in load order so same-(blk,win) windows match across cores
    # and consecutive windows have similar degrees (conv2 octet padding).
    order = np.argsort(-deg, kind="stable")
    bin_members = [order[j * WIN:(j + 1) * WIN] for j in range(nbins)]
    binload = np.array([deg[m].sum() if len(m) else 0 for m in bin_members],
                       np.int64)
    brank = np.argsort(-binload, kind="stable")
    slot_of = np.zeros(n, np.int64)     # global position of node v
    for r, j in enumerate(brank):
        rnd, k = divmod(r, ncores)
        core = k if rnd % 2 == 0 else ncores - 1 - k
        base = core * nlocp + rnd * WIN
        for s_, v in enumerate(bin_members[j]):
            slot_of[v] = base + s_

    loops = np.arange(n, dtype=np.int64)
    s_all = np.concatenate([src, loops])
    d_all = np.concatenate([dst, loops])
    dpos = slot_of[d_all]               # dest slot position
    spos = slot_of[s_all]               # source row in the h~ table
    core = dpos // nlocp
    rem = dpos - core * nlocp
    blk = rem // P
    win = (rem % P) // WIN
    wloc = rem % WIN

    cnt = np.zeros((ncores, nblk, NWIN), np.int64)
    np.add.at(cnt, (core, blk, win), 1)
    T = np.maximum(1, -(-cnt // P)).max(axis=0)          # [nblk, NWIN]
    tb = T.sum(axis=1)
    slot_base_bw = np.zeros((nblk, NWIN), np.int64)
    flat = T.reshape(-1)
    slot_base_bw.reshape(-1)[1:] = np.cumsum(flat)[:-1]
    tot = int(flat.sum())

    # order edges by (core, blk, win); sequence within group -> tile/lane
    key = (core * nblk + blk) * NWIN + win
    order_e = np.argsort(key, kind="stable")
    c_o, b_o, w_o, wl_o, sp_o = (core[order_e], blk[order_e], win[order_e],
                                 wloc[order_e], spos[order_e])
    key_o = key[order_e]
    first = np.r_[True, key_o[1:] != key_o[:-1]]
    idx_arr = np.arange(len(key_o))
    grp_start = np.maximum.accumulate(np.where(first, idx_arr, 0))
    seq = idx_arr - grp_start
    tile_k = seq // P
    jj = seq % P
    slot = slot_base_bw[b_o, w_o] + tile_k
    assert (tile_k < T[b_o, w_o]).all()

    idx16 = np.zeros((ncores, tot * P), np.int16)
    idx16[c_o, slot * P + jj] = sp_o.astype(np.int16)
    S = np.zeros((ncores, P, tot * WIN), BF16)
    S[c_o, jj, slot * WIN + wl_o] = BF16(1.0)

    # wrapped int16 index layout: slot-major i -> [i % 16, i // 16],
    # replicated into all 8 GPSIMD core partition groups (HW reads per-core)
    idxw = np.tile(idx16.reshape(ncores, tot * 8, 16).transpose(0, 2, 1),
                   (1, 8, 1)).copy()

    # ---------------- conv2 scalar-gather structures ----------------
    # window slots, STRIPED: w_core = blk*NWIN + win in [0, nblk*NWIN);
    # group g = w_core % NGRP handles slot s = w_core // NGRP. Striping makes
    # slot s pair CONSECUTIVE windows (8s..8s+7) across groups, which have
    # similar degrees under the degree-sorted snake deal -> minimal K padding.
    nwin_core = nblk * NWIN
    assert nwin_core % NGRP == 0
    wpg = nwin_core // NGRP            # windows (slots) per group
    zero_slot = ncores * nlocp          # zero entry appended to the z table

    # per-(core, window, lane) in-edge source lists
    # K[s] = max over (core, group, lane) of list length at window-slot s
    wcore_o = b_o * NWIN + w_o
    from collections import defaultdict
    lists = defaultdict(list)
    ne = len(c_o)
    for i in range(ne):
        lists[(int(c_o[i]), int(wcore_o[i]), int(wl_o[i]))].append(int(sp_o[i]))

    K = np.zeros(wpg, np.int64)
    for (c, w, l), ls in lists.items():
        s = w // NGRP
        K[s] = max(K[s], len(ls))
    L = int(32 * K.sum())
    while L % 16 != 0:
        K[int(np.argmin(K))] += 1
        L = int(32 * K.sum())

    # idx sequence per (core, group): slots s-major, lanes, then k
    idx2 = np.full((ncores, NGRP, L), zero_slot, np.int16)
    off = np.zeros(wpg + 1, np.int64)
    off[1:] = np.cumsum(32 * K)
    for (c, w, l), ls in lists.items():
        s, g = divmod(w, NGRP)
        base = off[s] + l * K[s]
        idx2[c, g, base:base + len(ls)] = np.asarray(ls, np.int16)
    # wrapped into 16 partitions per group: element i -> [16g + i%16, i//16]
    idx2w = idx2.reshape(ncores, NGRP, L // 16, 16).transpose(0, 1, 3, 2) \
                .reshape(ncores, NGRP * 16, L // 16).copy()

    return dict(
        dis=dis, nloc=nlocp, nblk=nblk, nlocp=nlocp, tot=tot,
        T=T, tb=tb, idxw=idxw, S=S, slot_of=slot_of,
        K=K, L=L, wpg=wpg, idx2w=idx2w, zero_slot=zero_slot,
    )


# ----------------------------------------------------------------------------
# Bass program
# ----------------------------------------------------------------------------

def build_program(n, f, ncores, nblk, nlocp, tot, T, tb, nloc, K, L, wpg):
    import concourse.bacc as bacc
    import concourse.mybir as mybir
    import concourse.tile as tile

    fpad = -(-f // P) * P          # bf16 row padded so 2*fpad % 256 == 0
    kch = fpad // P                # contraction chunks for x @ W1
    ag_rows = ncores * nlocp
    ztab_n = ag_rows + 2           # z table + zero slot (padded to even)
    dpg = nlocp // NGRP            # dests per group (e.g. 320)
    dt = mybir.dt
    Alu = mybir.AluOpType
    Act = mybir.ActivationFunctionType

    nc = bacc.Bacc("TRN2", target_bir_lowering=False, debug=False,
                   num_devices=ncores)

    xT_in = nc.dram_tensor("xT", [kch * P, nlocp], dt.bfloat16, kind="ExternalInput")
    W1_in = nc.dram_tensor("W1p", [kch * P, f], dt.bfloat16, kind="ExternalInput")
    vecs_in = nc.dram_tensor("vecs", [6, f], dt.float32, kind="ExternalInput")
    scal_in = nc.dram_tensor("scal", [1, 8], dt.float32, kind="ExternalInput")
    dis_in = nc.dram_tensor("disb", [P, nblk], dt.float32, kind="ExternalInput")
    disw_in = nc.dram_tensor("disw", [WIN, nblk * NWIN], dt.float32, kind="ExternalInput")
    disg_in = nc.dram_tensor("disg", [P, dpg], dt.float32, kind="ExternalInput")
    idx_in = nc.dram_tensor("idxw", [P, tot * 8], dt.int16, kind="ExternalInput")
    idx2_in = nc.dram_tensor("idx2w", [P, L // 16], dt.int16, kind="ExternalInput")
    S_in = nc.dram_tensor("S", [P, tot * WIN], dt.bfloat16, kind="ExternalInput")
    out_ext = nc.dram_tensor("out", [nloc, 1], dt.float32, kind="ExternalOutput")

    shared = "Shared" if ncores > 4 else "Local"
    h_loc = nc.dram_tensor("h_loc", [nlocp, fpad], dt.bfloat16)
    h_ag = nc.dram_tensor("h_ag", [ag_rows, fpad], dt.bfloat16, addr_space=shared)
    z_loc = nc.dram_tensor("z_loc", [nlocp, 1], dt.float32)
    z_ag = nc.dram_tensor("z_ag", [ag_rows, 1], dt.float32, addr_space=shared)

    rg = [list(range(ncores))]

    with tile.TileContext(nc) as tc:
        with (
            tc.tile_pool(name="const", bufs=1) as cp,
            tc.tile_pool(name="work", bufs=3) as wp,
            tc.tile_pool(name="msgp", bufs=3) as mp,
            tc.tile_pool(name="psum", bufs=8, space="PSUM") as pp,
        ):
            # ---------------- constants ----------------
            xT_sb = cp.tile([P, kch, nlocp], dt.bfloat16)
            nc.sync.dma_start(out=xT_sb[:], in_=xT_in.ap().rearrange("(k p) n -> p k n", p=P))
            W1_sb = cp.tile([P, kch, f], dt.bfloat16)
            nc.sync.dma_start(out=W1_sb[:], in_=W1_in.ap().rearrange("(k p) n -> p k n", p=P))
            scal = cp.tile([1, 8], dt.float32)
            nc.sync.dma_start(out=scal[:], in_=scal_in[:])
            dis_sb = cp.tile([P, nblk], dt.float32)
            nc.sync.dma_start(out=dis_sb[:], in_=dis_in[:])
            disw_sb = cp.tile([WIN, nblk * NWIN], dt.float32)
            nc.sync.dma_start(out=disw_sb[:], in_=disw_in[:])
            disg_sb = cp.tile([P, dpg], dt.float32)
            nc.sync.dma_start(out=disg_sb[:], in_=disg_in[:])
            idx_sb = cp.tile([P, tot * 8], dt.int16)
            nc.sync.dma_start(out=idx_sb[:], in_=idx_in[:])
            idx2_sb = cp.tile([P, L // 16], dt.int16)
            nc.sync.dma_start(out=idx2_sb[:], in_=idx2_in[:])
            S_sb = cp.tile([P, tot * WIN], dt.bfloat16)
            nc.sync.dma_start(out=S_sb[:], in_=S_in[:])

            ones = cp.tile([1, P], dt.float32)
            nc.vector.memset(ones[:], 1.0)

            # zero the padded tail columns of the gather table once
            zpad = cp.tile([P, fpad - f], dt.bfloat16)
            nc.vector.memset(zpad[:], 0.0)
            for nb in range(nblk):
                nc.sync.dma_start(out=h_loc[nb * P:(nb + 1) * P, f:], in_=zpad[:])

            # copy each param row to its own partition-0 tile
            vrow = []
            for i in range(6):
                r = cp.tile([1, f], dt.float32, tag=f"vrow{i}")
                nc.sync.dma_start(out=r[:], in_=vecs_in[i:i + 1, :])
                vrow.append(r)

            # k1 = g1 / sqrt(rv1 + eps); t1 = beta1 - rm1 * k1      [1, f]
            k1 = cp.tile([1, f], dt.float32)
            t1 = cp.tile([1, f], dt.float32)
            tmp = cp.tile([1, f], dt.float32)
            nc.vector.tensor_scalar_add(tmp[:], vrow[4][:], EPS)
            nc.scalar.sqrt(tmp[:], tmp[:])
            nc.vector.reciprocal(tmp[:], tmp[:])
            nc.vector.tensor_tensor(out=k1[:], in0=tmp[:], in1=vrow[1][:], op=Alu.mult)
            nc.vector.tensor_tensor(out=tmp[:], in0=vrow[3][:], in1=k1[:], op=Alu.mult)
            nc.vector.tensor_tensor(out=t1[:], in0=vrow[2][:], in1=tmp[:], op=Alu.subtract)

            # k2 = g2 / sqrt(rv2 + eps); t2 = beta2 - rm2 * k2; pack [1,4]: b2,k2,t2
            sc_row = cp.tile([1, 4], dt.float32)
            nc.vector.memset(sc_row[:], 0.0)
            stmp = cp.tile([1, 1], dt.float32)
            nc.vector.tensor_copy(out=sc_row[:, 0:1], in_=scal[:, 0:1])           # b2
            nc.vector.tensor_scalar_add(stmp[:], scal[:, 4:5], EPS)
            nc.scalar.sqrt(stmp[:], stmp[:])
            nc.vector.reciprocal(stmp[:], stmp[:])
            nc.vector.tensor_tensor(out=sc_row[:, 1:2], in0=stmp[:], in1=scal[:, 1:2], op=Alu.mult)  # k2
            nc.vector.tensor_tensor(out=stmp[:], in0=scal[:, 3:4], in1=sc_row[:, 1:2], op=Alu.mult)
            nc.vector.tensor_tensor(out=sc_row[:, 2:3], in0=scal[:, 2:3], in1=stmp[:], op=Alu.subtract)  # t2

            # replicate rows across 128 partitions via ones-matmul
            def replicate(row_ap, width):
                ps = pp.tile([P, width], dt.float32, tag="ps")
                nc.tensor.matmul(out=ps[:], lhsT=ones[:], rhs=row_ap, start=True, stop=True)
                sb = cp.tile([P, width], dt.float32, tag=f"rep{replicate.i}")
                replicate.i += 1
                nc.vector.tensor_copy(out=sb[:], in_=ps[:])
                return sb
            replicate.i = 0

            B1rep = replicate(vrow[0][:], f)
            K1rep = replicate(k1[:], f)
            T1rep = replicate(t1[:], f)
            W2rep = replicate(vrow[5][:], f)
            SCrep = replicate(sc_row[:], 4)      # cols: b2, k2, t2

            # ---------------- phase 1: h~ = dis * (x @ W1) ----------------
            for nb in range(nblk):
                ps = pp.tile([P, f], dt.float32, tag="ps")
                for kc in range(kch):
                    nc.tensor.matmul(
                        out=ps[:],
                        lhsT=xT_sb[:, kc, nb * P:(nb + 1) * P],
                        rhs=W1_sb[:, kc, :],
                        start=(kc == 0), stop=(kc == kch - 1),
                    )
                hb = wp.tile([P, f], dt.bfloat16, tag="hb")
                nc.vector.tensor_scalar_mul(hb[:], ps[:], dis_sb[:, nb:nb + 1])
                nc.sync.dma_start(out=h_loc[nb * P:(nb + 1) * P, :f], in_=hb[:])

            nc.gpsimd.collective_compute(
                "AllGather", Alu.bypass, replica_groups=rg,
                ins=[h_loc[:]], outs=[h_ag[:]],
            )

            # ---------------- conv1: gather + aggregate + fused epilogue ----
            # z by (lane, window): z_sb[l, b*NWIN+w] = z of slot b*128+w*32+l
            z_sb = cp.tile([WIN, nblk * NWIN], dt.float32)

            TCAP = 16           # max tiles per dma_gather (2048 idxs)
            for b in range(nblk):
                base = int(T[:b].sum()) if b else 0
                ntile = int(tb[b])
                # gather in pieces of <= TCAP tiles
                pieces = []          # (start_slot_abs, len, tile)
                done = 0
                while done < ntile:
                    plen = min(TCAP, ntile - done)
                    mseg = mp.tile([P, plen, fpad], dt.bfloat16, tag="msg")
                    nc.gpsimd.dma_gather(
                        out_ap=mseg[:],
                        in_ap=h_ag[:],
                        idxs_ap=idx_sb[:, (base + done) * 8:(base + done + plen) * 8],
                        num_idxs=plen * P,
                        num_idxs_reg=plen * P,
                        elem_size=fpad,
                        single_packet=False,
                    )
                    pieces.append((base + done, plen, mseg))
                    done += plen
                msg = pieces[-1][2]   # for the dummy ap_gather below
                slot = base
                pi = 0
                for w in range(NWIN):
                    tw = int(T[b, w])
                    pw = pp.tile([WIN, f], dt.float32, tag="ps")
                    for k in range(tw):
                        while slot >= pieces[pi][0] + pieces[pi][1]:
                            pi += 1
                        pstart, _, mseg = pieces[pi]
                        nc.tensor.matmul(
                            out=pw[:],
                            lhsT=S_sb[:, slot * WIN:(slot + 1) * WIN],
                            rhs=mseg[:, slot - pstart, :f],
                            start=(k == 0), stop=(k == tw - 1),
                        )
                        slot += 1
                    dw = disw_sb[:, b * NWIN + w:b * NWIN + w + 1]
                    u = wp.tile([WIN, f], dt.float32, tag="u")
                    # u = (psum * dis_d) + b1 ; relu ; u = u*k1 + t1 ; relu
                    nc.vector.scalar_tensor_tensor(
                        out=u[:], in0=pw[:], scalar=dw, in1=B1rep[:WIN, :],
                        op0=Alu.mult, op1=Alu.add)
                    nc.scalar.activation(u[:], u[:], Act.Relu)
                    nc.vector.tensor_tensor(out=u[:], in0=u[:], in1=K1rep[:WIN, :], op=Alu.mult)
                    nc.vector.tensor_tensor(out=u[:], in0=u[:], in1=T1rep[:WIN, :], op=Alu.add)
                    nc.scalar.activation(u[:], u[:], Act.Relu)
                    # z = dis_d * (u @ W2): mult + row-reduce
                    sc = wp.tile([WIN, f], dt.float32, tag="sc")
                    nc.vector.tensor_tensor(out=sc[:], in0=u[:], in1=W2rep[:WIN, :], op=Alu.mult)
                    zb = wp.tile([WIN, 1], dt.float32, tag="zb")
                    nc.vector.tensor_reduce(out=zb[:], in_=sc[:], axis=mybir.AxisListType.X, op=Alu.add)
                    nc.vector.tensor_scalar_mul(
                        z_sb[:, b * NWIN + w:b * NWIN + w + 1], zb[:], dw)

            # dummy ap_gather tied to the last msg tile: forces the Pool ucode
            # library switch (mlp -> ap_gather, ~220us) to start here, hidden
            # under the conv1 epilogue tail + z AllGather.
            zidx = cp.tile([P, 1], dt.int16)
            nc.vector.memset(zidx[:], 0)
            dummy = cp.tile([P, 16, 2], dt.bfloat16)
            nc.gpsimd.ap_gather(
                out_ap=dummy[:],
                in_ap=msg[:, 0, :].rearrange("p (n d) -> p n d", d=2),
                idxs_ap=zidx[:], channels=P, num_elems=192, d=2, num_idxs=16,
            )

            nc.sync.dma_start(
                out=z_loc.ap().rearrange("(bw l) one -> l (bw one)", l=WIN),
                in_=z_sb[:])

            nc.gpsimd.collective_compute(
                "AllGather", Alu.bypass, replica_groups=rg,
                ins=[z_loc[:]], outs=[z_ag[:]],
            )

            # ---------------- conv2: scalar gather + segmented reduce -------
            ztab = cp.tile([P, ztab_n], dt.float32)
            nc.sync.dma_start(out=ztab[0:1, :ag_rows],
                              in_=z_ag.ap().rearrange("n one -> one n"))
            nc.gpsimd.partition_broadcast(ztab[:, :ag_rows], ztab[0:1, :ag_rows],
                                          channels=P)
            nc.vector.memset(ztab[:, ag_rows:], 0.0)

            o2 = cp.tile([P, dpg], dt.float32)
            off = 0
            for s in range(wpg):
                ks = int(K[s])
                zg = wp.tile([P, 32 * ks], dt.float32, tag="zg")
                nc.gpsimd.ap_gather(
                    out_ap=zg[:], in_ap=ztab[:],
                    idxs_ap=idx2_sb[:, off // 16:(off + 32 * ks) // 16],
                    channels=P, num_elems=ztab_n, d=1, num_idxs=32 * ks,
                )
                nc.vector.tensor_reduce(
                    out=o2[:, s * WIN:(s + 1) * WIN],
                    in_=zg[:].rearrange("p (l k) -> p l k", k=ks),
                    axis=mybir.AxisListType.X, op=Alu.add)
                off += 32 * ks

            # epilogue: v = k2*(dis_d*agg + b2) + t2 ; relu ; sigmoid
            nc.vector.tensor_tensor(out=o2[:], in0=o2[:], in1=disg_sb[:], op=Alu.mult)
            nc.vector.tensor_scalar_add(o2[:], o2[:], SCrep[:, 0:1])
            nc.vector.tensor_scalar_mul(o2[:], o2[:], SCrep[:, 1:2])
            nc.vector.tensor_scalar_add(o2[:], o2[:], SCrep[:, 2:3])
            nc.scalar.activation(o2[:], o2[:], Act.Relu)
            nc.scalar.activation(o2[:], o2[:], Act.Sigmoid)

            out_gsl = out_ext.ap().rearrange(
                "(s g l) one -> g s (l one)", s=wpg, g=NGRP)
            for g in range(NGRP):
                nc.sync.dma_start(
                    out=out_gsl[g:g + 1, :, :],
                    in_=o2[16 * g:16 * g + 1, :])

    nc.compile()
    return nc


# ----------------------------------------------------------------------------
# Full pipeline
# ----------------------------------------------------------------------------

def make_inputs(x, W1, b1, g1, beta1, rm1, rv1, W2, b2, g2, beta2, rm2, rv2,
                pre, ncores):
    n, f = x.shape
    nlocp, nblk = pre["nlocp"], pre["nblk"]
    slot_of = pre["slot_of"]
    kch = -(-f // P)
    fpk = kch * P
    dpg = nlocp // NGRP

    W1p = np.zeros((fpk, f), BF16)
    W1p[:f, :] = W1.astype(BF16)
    vecs = np.stack([b1, g1, beta1, rm1, rv1, W2[:, 0]]).astype(np.float32)
    scal = np.zeros((1, 8), np.float32)
    scal[0, :5] = [b2[0], g2[0], beta2[0], rm2[0], rv2[0]]

    dis = pre["dis"]
    core_of = slot_of // nlocp
    local = slot_of - core_of * nlocp
    in_maps = []
    for c in range(ncores):
        sel = core_of == c
        loc = local[sel]
        xT = np.zeros((fpk, nlocp), BF16)
        xT[:f, loc] = x[sel].T.astype(BF16)
        db = np.zeros(nlocp, np.float32)
        db[loc] = dis[sel]
        disb = db.reshape(nblk, P).T.copy()
        disw = db.reshape(nblk * (P // WIN), WIN).T.copy()
        # striped groups: group g's dests are windows g, g+8, g+16, ... in
        # slot order; replicate each group's row across its 16 partitions
        W = db.reshape(nblk * NWIN, WIN)
        disg = np.zeros((P, dpg), np.float32)
        for g in range(NGRP):
            disg[16 * g:16 * (g + 1), :] = W[g::NGRP].reshape(1, dpg)
        in_maps.append({
            "xT": xT, "W1p": W1p, "vecs": vecs, "scal": scal,
            "disb": disb, "disw": disw, "disg": disg,
            "idxw": pre["idxw"][c], "idx2w": pre["idx2w"][c], "S": pre["S"][c],
        })
    return in_maps


def _install_ntff_hook():
    """bass_utils wants antenv.axon_hooks for trace=True under axon; this
    container's antenv lacks it. Inject a shim backed by the boot helper."""
    import sys, types
    if "antenv.axon_hooks" in sys.modules:
        return
    try:
        from trn_agent_boot.trn_boot import _ntff_profile_via_ctypes
        hook = _ntff_profile_via_ctypes("/opt/axon/libaxon_pjrt.so")
    except Exception:
        hook = None
    mod = types.ModuleType("antenv.axon_hooks")
    mod.get_axon_ntff_profile_hook = lambda: hook
    mod.set_axon_ntff_profile_hook = lambda h: None
    sys.modules["antenv.axon_hooks"] = mod


def run(inputs, ncores=8, trace=False, tmpdir=None):
    from concourse.bass_utils import run_bass_kernel_spmd
    if trace:
        _install_ntff_hook()

    x = np.asarray(inputs["x"])
    n, f = x.shape
    pre = preprocess(np.asarray(inputs["edge_index"]), n, ncores)
    nc = build_program(n, f, ncores, pre["nblk"], pre["nlocp"], pre["tot"],
                       pre["T"], pre["tb"], pre["nloc"],
                       pre["K"], pre["L"], pre["wpg"])
    in_maps = make_inputs(
        x, *(np.asarray(inputs[k]) for k in
             ["W1", "b1", "g1", "beta1", "rm1", "rv1",
              "W2", "b2", "g2", "beta2", "rm2", "rv2"]),
        pre, ncores)
    res = run_bass_kernel_spmd(nc, in_maps, list(range(ncores)), trace=trace,
                               tmpdir=tmpdir)
    allout = np.concatenate([res.results[c]["out"] for c in range(ncores)], axis=0)
    out = allout[pre["slot_of"]]
    return out, res, pre, nc


_CACHE = {}


def kernel(**inputs) -> np.ndarray:
    out, _res, _pre, _nc = run(inputs, ncores=8, trace=False)
    return out.astype(np.float32)
